# revision 1
# baseline (speedup 1.0000x reference)
"""Distributed Trainium2 Bass kernel for nn_AttentionD_12412455485977.

3D-windowed multi-head attention with relative-position bias:
  qkv = x @ w_qkv ; per-head attention with bias gathered from rel_table
  via the static relative-position index; out = attn_out @ w_out + b_out.

Sharding: head-parallel. Core c computes head c for both batches and the
partial out-projection attn_out_h @ w_out[h]; the host sums the 8 partial
[2*2048, 128] results (the natural unshard of a head-sharded contraction).
b_out is added on core 0 only (scaled by the softmax denominator so the
in-kernel normalization leaves it intact).

Bias trick: bias[i, j] depends only on (pos_i - pos_j); with n ordered
z-major, the [2048, 2048] per-head bias matrix is block-Toeplitz over z
with 256x256 blocks indexed by dz = zi - zj.  exp factorizes across the
softmax: exp(s + b) = exp(s) * exp(b), so the kernel multiplies exp(scores)
by host-precomputed exp(bias) slabs that live entirely in SBUF.
"""

import os
import sys

import numpy as np

for _p in ("/opt/trn_rl_repo", "/root/.axon_site/_ro/trn_rl_repo"):
    if os.path.isdir(_p) and _p not in sys.path:
        sys.path.append(_p)

import ml_dtypes  # noqa: E402
import concourse.bass as bass  # noqa: E402
import concourse.tile as tile  # noqa: E402
from concourse import bacc, mybir  # noqa: E402
from concourse.bass_utils import run_bass_kernel_spmd  # noqa: E402

BF16 = mybir.dt.bfloat16
F32 = mybir.dt.float32
NPBF16 = ml_dtypes.bfloat16

B = 2            # batches
N = 2048         # tokens per batch (= 8*16*16, z-major)
C = 128          # channels
HEADS = 8
DH = 32          # head dim
D3, H3, W3 = 8, 16, 16
NCORES = 8

# ---------------------------------------------------------------------------
# host-side static index table for the exp(bias) slabs
# ---------------------------------------------------------------------------
# bias7[p, k*2048 + g*512 + ih] multiplies exp(scores^T) for the step with
# chunk/group offset k = ic - t + 3:
#   scores^T[j, i] tile with j = (4t+g)*128 + p, i = ic*512 + ih.
# biasT[j, i] = T[(zi-zj+7)*961 + (dy+15)*31 + (dx+15)]


def _bias7_index() -> np.ndarray:
    kk = np.arange(7)[:, None, None, None]
    gg = np.arange(4)[None, :, None, None]
    pp = np.arange(128)[None, None, :, None]
    ii = np.arange(512)[None, None, None, :]
    a = 2 * kk + 1 + ii // 256 - gg // 2          # zi - zj + 7
    pj = (gg % 2) * 128 + pp
    pi = ii % 256
    dy = pi // 16 - pj // 16 + 15
    dx = pi % 16 - pj % 16 + 15
    return (a * 961 + dy * 31 + dx).astype(np.int32)  # [7, 4, 128, 512]


_IDX7 = _bias7_index()

# ---------------------------------------------------------------------------
# device graph
# ---------------------------------------------------------------------------


def _build():
    nc = bacc.Bacc(None, target_bir_lowering=False, debug=False)

    xt_e = nc.declare_dram_parameter("xt", [C, B * N], BF16, isOutput=False)
    w3_e = nc.declare_dram_parameter("w3", [C, 96], BF16, isOutput=False)
    # [0:32, 0:128] = w_out head slice; [32, 0:128] = b_out (core 0 only);
    # col 128 = denominator passthrough (e_32) so the projection matmul also
    # transposes the softmax denominator into partition-major layout.
    waug_e = nc.declare_dram_parameter("waug", [DH + 1, C + 1], F32, isOutput=False)
    bias7_e = nc.declare_dram_parameter("bias7", [128, 7 * 2048], BF16, isOutput=False)
    out_e = nc.declare_dram_parameter("out", [B * N, C], F32, isOutput=True)

    with tile.TileContext(nc) as tc:
        with tc.tile_pool(name="persist", bufs=1) as persist:
            # DMA issue order tracks the critical path: phase-1 batch-0 needs
            # w3 + xt[0] immediately; everything else is deferred below.
            w3 = persist.tile([C, 96], BF16)
            nc.sync.dma_start(w3[:], w3_e[:])
            waug = persist.tile([DH + 1, C + 1], F32)
            nc.sync.dma_start(waug[:], waug_e[:])
            xt = [persist.tile([C, N], BF16, tag=f"xt{b}", name=f"xt{b}")
                  for b in range(B)]
            xt0_dma = nc.sync.dma_start(xt[0][:], xt_e[:, 0:N])
            bias7 = [persist.tile([128, 2048], BF16, tag=f"bias{k}",
                                  name=f"bias{k}") for k in range(7)]
            # chain the bulk loads behind the phase-1-critical xt[0] transfer
            # (slab k is first read at half-step k..; the serial chain keeps
            # each ~1.5us transfer off the critical DMA path but early enough)
            slab_dma = {}
            for k in range(7):
                slab_dma[k] = nc.gpsimd.dma_start(
                    bias7[k][:], bias7_e[:, k * 2048:(k + 1) * 2048])
            xt1_dma = nc.sync.dma_start(xt[1][:], xt_e[:, N:2 * N])
            # slab 3 (first used) may start once w3 is in; the rest are
            # gated on the per-batch shuffle DMAs below so the serialized
            # transfer path serves the latency-critical loads first.
            tile.add_dep_helper(slab_dma[3].ins, xt0_dma.ins, sync=True,
                                reason="slab3 after xt0")
            tile.add_dep_helper(xt1_dma.ins, slab_dma[3].ins, sync=True,
                                reason="xt1 after slab3")
            # touch the Exp table so the ~2.7us ACT_TABLE_LOAD overlaps
            # phase 1 instead of gating the first real exp.
            scratch = persist.tile([128, 1], F32)
            nc.vector.memset(scratch[:], 0.0)
            nc.scalar.activation(scratch[:], scratch[:],
                                 mybir.ActivationFunctionType.Exp)

            # per-batch tiles so phase 2 for batch 0 only depends on batch-0 prep
            qkT = [persist.tile([64, N], BF16, tag=f"qkT{b}", name=f"qkT{b}") for b in range(B)]
            qT4 = [persist.tile([128, N], BF16, tag=f"qT4{b}", name=f"qT4{b}") for b in range(B)]
            kT4 = [persist.tile([128, 512], BF16, tag=f"kT4{b}", name=f"kT4{b}") for b in range(B)]
            vaug = [persist.tile([128, 16 * 33], BF16, tag=f"vaug{b}", name=f"vaug{b}") for b in range(B)]

            # ---- phase 1: qkv projections -------------------------------
            with tc.tile_pool(name="ph1", bufs=2, space="PSUM") as ph1:
                for b in range(B):
                    nc.vector.memset(vaug[b][:], 1.0)
                    for ch in range(4):
                        qk_ps = ph1.tile([64, 512], F32)
                        nc.tensor.matmul(qk_ps[:], lhsT=w3[:, 0:64],
                                         rhs=xt[b][:, ch * 512:(ch + 1) * 512],
                                         start=True, stop=True)
                        nc.vector.tensor_copy(qkT[b][:, ch * 512:(ch + 1) * 512],
                                              qk_ps[:])
                    for tt in range(4):
                        v_ps = ph1.tile([128, 128], F32)
                        for u in range(4):
                            nt = tt * 4 + u
                            nc.tensor.matmul(v_ps[:, u * 32:(u + 1) * 32],
                                             lhsT=xt[b][:, nt * 128:(nt + 1) * 128],
                                             rhs=w3[:, 64:96],
                                             start=True, stop=True)
                        dst = vaug[b][:, tt * 132:(tt + 1) * 132]
                        dst = dst.rearrange("p (f c) -> p f c", f=4)[:, :, 0:DH]
                        src = v_ps[:].rearrange("p (f c) -> p f c", f=4)
                        nc.vector.tensor_copy(dst, src)
                    # replicate q across partition groups; scatter k by j-tile
                    for g in range(4):
                        shf = nc.sync.dma_start(qT4[b][32 * g:32 * (g + 1), :],
                                                qkT[b][0:32, :])
                        src = qkT[b][32:64, :].rearrange(
                            "d (t g jj) -> d t g jj", t=4, g=4, jj=128)[:, :, g, :]
                        dst = kT4[b][32 * g:32 * (g + 1), :].rearrange(
                            "d (t jj) -> d t jj", t=4)
                        shf = nc.sync.dma_start(dst, src)
                    # release the next tranche of bias-slab transfers
                    for k in ((2, 1, 0) if b == 0 else (4, 5, 6)):
                        tile.add_dep_helper(slab_dma[k].ins, shf.ins, sync=True,
                                            reason="slabs after shuffles")

            # ---- phase 2: attention ------------------------------------
            # half-steps of 2 j-tiles (2 PSUM banks) so the exp eviction of
            # one buffer overlaps the score matmuls filling the other.
            with (
                tc.tile_pool(name="score", bufs=2, space="PSUM") as score_pool,
                tc.tile_pool(name="outps", bufs=2, space="PSUM") as out_pool,
                tc.tile_pool(name="proj", bufs=1, space="PSUM") as proj_pool,
                tc.tile_pool(name="sb2", bufs=3) as sb2,
                tc.tile_pool(name="sb3", bufs=2) as sb3,
            ):
                def epilogue(b, ic, out_ps):
                    # evict PV psum, project (incl. denominator column and
                    # denominator-scaled b_out), normalize, store.
                    outT = sb3.tile([DH + 1, 512], F32, tag="outT", name="outT")
                    nc.vector.tensor_copy(outT[:], out_ps[:])
                    for half in range(2):
                        proj_ps = proj_pool.tile([128, 2 * (C + 1)], F32,
                                                 tag=f"proj{half}", bufs=1,
                                                 name=f"proj{half}")
                        for u in range(2):
                            it = 2 * half + u
                            nc.tensor.matmul(
                                proj_ps[:, u * (C + 1):(u + 1) * (C + 1)],
                                lhsT=outT[:, it * 128:(it + 1) * 128],
                                rhs=waug[:], start=True, stop=True)
                        pv = proj_ps[:].rearrange("p (f c) -> p f c", f=2)
                        recip = sb3.tile([128, 2], F32, tag=f"recip{half}",
                                         name=f"recip{half}")
                        nc.vector.reciprocal(recip[:],
                                             pv[:, :, C:C + 1].squeeze(2))
                        osb = sb3.tile([128, 2 * C], F32, tag=f"osb{half}",
                                       name=f"osb{half}")
                        ov = osb[:].rearrange("p (f c) -> p f c", f=2)
                        for u in range(2):
                            nc.vector.tensor_scalar_mul(
                                ov[:, u, :], pv[:, u, 0:C],
                                recip[:, u:u + 1])
                        for u in range(2):
                            it = 2 * half + u
                            row = b * N + ic * 512 + it * 128
                            nc.sync.dma_start(out_e[row:row + 128, :],
                                              ov[:, u, :])

                # software pipeline over global half-steps: emit step s's
                # score matmuls BEFORE step s-1's exp/mul/PV so the static
                # per-engine order keeps PE feeding ACT ahead of PV work.
                steps = []
                for b in range(B):
                    for ic in range(4):
                        for t in range(4):
                            for hh in range(2):
                                steps.append((b, ic, t, hh))
                out_ps_of = {}
                carry = None   # (emit_rest closure for step s-1)
                post = None    # epilogue closure for the chunk that ended
                for s, (b, ic, t, hh) in enumerate(steps):
                    if (t, hh) == (0, 0):
                        out_ps_of[(b, ic)] = out_pool.tile(
                            [DH + 1, 512], F32, name="out_ps", tag="out_ps")
                    score_ps = score_pool.tile([128, 1024], F32,
                                               name="score_ps", tag="score_ps")
                    for gg in range(2):
                        g = 2 * hh + gg
                        nc.tensor.matmul(
                            score_ps[:, gg * 512:(gg + 1) * 512],
                            lhsT=kT4[b][32 * g:32 * (g + 1),
                                        t * 128:(t + 1) * 128],
                            rhs=qT4[b][32 * g:32 * (g + 1),
                                       ic * 512:(ic + 1) * 512],
                            start=True, stop=True,
                            tile_position=(32 * g, 0))
                    if carry is not None:
                        carry()
                    if post is not None and (t, hh) >= (1, 0):
                        post()
                        post = None

                    def emit_rest(b=b, ic=ic, t=t, hh=hh, score_ps=score_ps):
                        expS = sb2.tile([128, 1024], BF16, tag="expS",
                                        name="expS")
                        nc.scalar.activation(expS[:], score_ps[:],
                                             mybir.ActivationFunctionType.Exp)
                        k7 = ic - t + 3
                        expT = sb2.tile([128, 1024], BF16, tag="expT",
                                        name="expT")
                        nc.vector.tensor_mul(
                            expT[:], expS[:],
                            bias7[k7][:, hh * 1024:(hh + 1) * 1024])
                        out_ps = out_ps_of[(b, ic)]
                        for gg in range(2):
                            jt = 4 * t + 2 * hh + gg
                            nc.tensor.matmul(
                                out_ps[:],
                                lhsT=vaug[b][:, jt * 33: jt * 33 + 33],
                                rhs=expT[:, gg * 512:(gg + 1) * 512],
                                start=(t == 0 and hh == 0 and gg == 0),
                                stop=(t == 3 and hh == 1 and gg == 1),
                                skip_group_check=True)

                    carry = emit_rest
                    if (t, hh) == (3, 1):
                        post = (lambda b=b, ic=ic:
                                epilogue(b, ic, out_ps_of[(b, ic)]))
                carry()
                post()

    nc.compile()
    return nc


_NC = None


def _get_nc():
    global _NC
    if _NC is None:
        _NC = _build()
    return _NC


# ---------------------------------------------------------------------------
# host side
# ---------------------------------------------------------------------------


def _prep_in_maps(x, w_qkv, rel_table, w_out, b_out):
    x = np.asarray(x, np.float32)
    w_qkv = np.asarray(w_qkv, np.float32)
    rel_table = np.asarray(rel_table, np.float32)
    w_out = np.asarray(w_out, np.float32)
    b_out = np.asarray(b_out, np.float32)

    scale = DH ** -0.5
    xt = np.ascontiguousarray(x.transpose(2, 0, 1).reshape(C, B * N)).astype(NPBF16)

    in_maps = []
    for hc in range(NCORES):
        w3 = np.concatenate([
            w_qkv[:, hc * DH:(hc + 1) * DH] * scale,
            w_qkv[:, 256 + hc * DH: 256 + (hc + 1) * DH],
            w_qkv[:, 512 + hc * DH: 512 + (hc + 1) * DH],
        ], axis=1).astype(NPBF16)
        waug = np.zeros((DH + 1, C + 1), np.float32)
        waug[0:DH, 0:C] = w_out[hc * DH:(hc + 1) * DH, :]
        if hc == 0:
            waug[DH, 0:C] = b_out
        waug[DH, C] = 1.0
        bias7 = np.exp(rel_table[:, hc][_IDX7])            # [7, 4, 128, 512]
        bias7 = np.ascontiguousarray(
            bias7.transpose(2, 0, 1, 3).reshape(128, 7 * 2048)).astype(NPBF16)
        in_maps.append({
            "xt": xt,
            "w3": np.ascontiguousarray(w3),
            "waug": waug,
            "bias7": bias7,
        })
    return in_maps


def _run(in_maps, **kwargs):
    nc = _get_nc()
    return run_bass_kernel_spmd(nc, in_maps, core_ids=list(range(NCORES)), **kwargs)


def kernel(x, w_qkv, rel_table, w_out, b_out, d=None, h=None, w=None):
    in_maps = _prep_in_maps(x, w_qkv, rel_table, w_out, b_out)
    res = _run(in_maps)
    acc = np.zeros((B * N, C), np.float64)
    for i in range(NCORES):
        acc += res.results[i]["out"].astype(np.float64)
    return acc.reshape(B, N, C).astype(np.float32)



# revision 16
# speedup vs baseline: 1.2474x; 1.2474x over previous
"""Distributed Trainium2 Bass kernel for nn_AttentionD_12412455485977.

3D-windowed multi-head attention with relative-position bias:
  qkv = x @ w_qkv ; per-head attention with bias from rel_table via the
  static relative-position index; out = attn_out @ w_out + b_out.

Sharding: head-parallel. Core c computes head c for both batches and the
partial out-projection attn_out_h @ w_out[h]; the host sums the 8 partial
[2*2048, 128] results.  b_out is added on core 0 only (scaled by the
softmax denominator so the in-kernel normalization leaves it intact).

v2 design (per step = one [128 j, 1024 i] score tile, 64 steps/core):
- scores: fp8(e4m3) q/k with a DoubleRow matmul whose pair dim is a
  stride-0 broadcast (each product counted twice; 0.5 folded into the
  host-side qk scale).  2x PE throughput vs bf16.
- bias: added in log space directly into the score PSUM by a second
  DoubleRow matmul, lhsT = one-hot "identity" (row j hot at (j%64, j//64)),
  rhs = host-packed fp8 log-bias slab.  No separate element-wise multiply.
- exp: split between the ACT engine (Exp activation, PSUM->SBUF) and
  GPSIMD (pow(e, x) tensor_tensor; needs a DVE PSUM->SBUF copy first
  since GPSIMD cannot access PSUM).
- PV: "flipped" layout — out_acc[i, d] += expT[:, i-tile].T @ vaug[j, 33]
  so each matmul streams only 33 output rows; per-chunk PE transposes
  (via identity) restore [d, i] for the fused projection + denominator
  normalization of the baseline.
- bias slab relative-position structure: bias[i, j] depends only on
  (pos_i - pos_j); with n z-major the per-head bias matrix is block-
  Toeplitz over z, indexed by dz = zi - zj (7 slabs).
"""

import os
import sys

import numpy as np

for _p in ("/opt/trn_rl_repo", "/root/.axon_site/_ro/trn_rl_repo"):
    if os.path.isdir(_p) and _p not in sys.path:
        sys.path.append(_p)

import ml_dtypes  # noqa: E402
import concourse.bass as bass  # noqa: E402
import concourse.tile as tile  # noqa: E402
from concourse import bacc, mybir  # noqa: E402
from concourse.bass_utils import run_bass_kernel_spmd  # noqa: E402

BF16 = mybir.dt.bfloat16
F32 = mybir.dt.float32
FP8 = mybir.dt.float8e4
NPBF16 = ml_dtypes.bfloat16
NPFP8 = ml_dtypes.float8_e4m3

B = 2            # batches
N = 2048         # tokens per batch (= 8*16*16, z-major)
C = 128          # channels
HEADS = 8
DH = 32          # head dim
NCORES = 8
DR = mybir.MatmulPerfMode.DoubleRow
EXPF = mybir.ActivationFunctionType.Exp

# which in-chunk step positions (0..7) run exp on GPSIMD instead of ACT
# exp on ACT everywhere: with 2-slot score lookahead the ACT engine
# self-paces gap-free, and GPSIMD offload's DVE-copy + PE-pacing overheads
# cost more than the ACT time they save.
POOL_POS = {c: () for c in range(8)}

# ---------------------------------------------------------------------------
# host-side static index table for the log-bias slabs
# ---------------------------------------------------------------------------
# slab[k7][j, g*512 + ih] = biasT[j, i] for the score tile with
# j = (4t+g)*128 + p, i = ic*512 + ih, where k7 = ic - t + 3.


def _bias7_index() -> np.ndarray:
    kk = np.arange(7)[:, None, None, None]
    gg = np.arange(4)[None, :, None, None]
    pp = np.arange(128)[None, None, :, None]
    ii = np.arange(512)[None, None, None, :]
    a = 2 * kk + 1 + ii // 256 - gg // 2          # zi - zj + 7
    pj = (gg % 2) * 128 + pp
    pi = ii % 256
    dy = pi // 16 - pj // 16 + 15
    dx = pi % 16 - pj % 16 + 15
    return (a * 961 + dy * 31 + dx).astype(np.int32)  # [7, 4, 128, 512]


_IDX7 = _bias7_index()

# ---------------------------------------------------------------------------
# device graph
# ---------------------------------------------------------------------------


def _dr2(ap, m):
    """stride-0 pair dim for DoubleRow: [32, m] -> [32, 2, m]."""
    return ap.unsqueeze(1).broadcast_to([32, 2, m])


def _build():
    nc = bacc.Bacc(None, target_bir_lowering=False, debug=False)

    xt_e = nc.declare_dram_parameter("xt", [C, B * N], BF16, isOutput=False)
    w3_e = nc.declare_dram_parameter("w3", [C, 96], BF16, isOutput=False)
    waug_e = nc.declare_dram_parameter("waug", [DH + 1, C + 1], BF16, isOutput=False)
    id_e = nc.declare_dram_parameter("ident", [128, 128], BF16, isOutput=False)
    lh_e = nc.declare_dram_parameter("lh", [128, 256], FP8, isOutput=False)
    slab_e = nc.declare_dram_parameter("slab", [128, 4 * 4096], FP8, isOutput=False)
    out_e = nc.declare_dram_parameter("out", [B * N, C], BF16, isOutput=True)
    dbg_e = None
    dbg2_e = None
    if os.environ.get("KDBG") == "1":
        dbg_e = nc.declare_dram_parameter("dbg", [32, 2 * N], BF16, isOutput=True)
        dbg2_e = nc.declare_dram_parameter("dbg2", [128, 132], BF16, isOutput=True)

    with tile.TileContext(nc) as tc:
        with tc.tile_pool(name="persist", bufs=1) as persist:
            # load order tracks the critical path: w3 + xt b0-ic0 + slab pt1
            # feed the first chunk; everything else trails.
            w3 = persist.tile([C, 96], BF16)
            nc.sync.dma_start(w3[:], w3_e[:])
            xt = [persist.tile([C, N], BF16, tag=f"xt{b}", name=f"xt{b}")
                  for b in range(B)]
            nc.sync.dma_start(xt[0][:, 0:512], xt_e[:, 0:512])
            lh = persist.tile([128, 256], FP8, name="lh")
            nc.sync.dma_start(lh[:], lh_e[:])
            slab = persist.tile([128, 4 * 4096], FP8, name="slab")
            # k7=3 (pt1 upper half) feeds the first step's bias
            nc.sync.dma_start(slab[64:128, 4096:8192], slab_e[64:128, 4096:8192])
            nc.sync.dma_start(xt[0][:, 512:2048], xt_e[:, 512:2048])
            nc.sync.dma_start(slab[0:64, 4096:8192], slab_e[0:64, 4096:8192])
            waug = persist.tile([DH + 1, C + 1], BF16)
            nc.sync.dma_start(waug[:], waug_e[:])
            nc.sync.dma_start(slab[:, 0:4096], slab_e[:, 0:4096])         # k7 0,1
            ident = persist.tile([128, 128], BF16, name="ident")
            nc.sync.dma_start(ident[:], id_e[:])
            nc.sync.dma_start(slab[:, 8192:12288], slab_e[:, 8192:12288])  # k7 4,5
            nc.sync.dma_start(xt[1][:], xt_e[:, N:2 * N])
            nc.sync.dma_start(slab[:, 12288:16384], slab_e[:, 12288:16384])  # k7 6

            # warm the Exp table off the critical path
            scratch = persist.tile([128, 1], F32)
            nc.vector.memset(scratch[:], 0.0)
            nc.scalar.activation(scratch[:], scratch[:], EXPF)

            base = persist.tile([128, 1024], BF16, name="base")
            nc.vector.memset(base[:], float(np.exp(1.0)))

            q8 = [persist.tile([32, N], BF16, tag=f"q8{b}", name=f"q8{b}")
                  for b in range(B)]
            k8 = [persist.tile([32, N], BF16, tag=f"k8{b}", name=f"k8{b}")
                  for b in range(B)]
            vaug = [persist.tile([128, 16 * 33], BF16, tag=f"vaug{b}",
                                 name=f"vaug{b}") for b in range(B)]

            with (
                tc.tile_pool(name="ph1", bufs=1, space="PSUM") as ph1,
                tc.tile_pool(name="score", bufs=2, space="PSUM") as score_pool,
                tc.tile_pool(name="outacc", bufs=2, space="PSUM") as out_pool,
                tc.tile_pool(name="epips", bufs=1, space="PSUM") as epi_pool,
                tc.tile_pool(name="sbexp", bufs=8) as sbexp,
                tc.tile_pool(name="sbcp", bufs=3) as sbcp,
                tc.tile_pool(name="sbepi", bufs=2) as sbepi,
            ):
                # ---- phase-1 emit helpers (called lazily inside the loop) --
                def em_qk(b, ic, which):
                    ps = ph1.tile([128, 512], F32, tag="ph1t", name="ph1t")
                    col = slice(0, 32) if which == "q" else slice(32, 64)
                    nc.tensor.matmul(ps[0:32, :], lhsT=w3[:, col],
                                     rhs=xt[b][:, ic * 512:(ic + 1) * 512],
                                     start=True, stop=True)
                    dst = (q8 if which == "q" else k8)[b][:, ic * 512:(ic + 1) * 512]
                    nc.vector.tensor_copy(dst, ps[0:32, :])

                def em_v4(b, g):
                    ps = ph1.tile([128, 512], F32, tag="ph1t", name="ph1t")
                    for u in range(4):
                        nt = 4 * g + u
                        nc.tensor.matmul(ps[:, u * 32:(u + 1) * 32],
                                         lhsT=xt[b][:, nt * 128:(nt + 1) * 128],
                                         rhs=w3[:, 64:96], start=True, stop=True)
                    dst = vaug[b][:, g * 132:(g + 1) * 132].rearrange(
                        "p (u c) -> p u c", u=4)[:, :, 0:32]
                    nc.vector.tensor_copy(
                        dst, ps[:, 0:128].rearrange("p (u c) -> p u c", u=4))

                # slot -> list of phase-1 closures (b0 ic0 q/k in prologue)
                ph1_sched = {}

                def sched(slot, fn):
                    ph1_sched.setdefault(slot, []).append(fn)

                for t in range(3):
                    sched(2 * t, lambda b=0, kc=t + 1: em_qk(b, kc, "k"))
                for g in range(4):
                    sched(2 * g, lambda g=g: em_v4(0, g))
                sched(5, lambda: em_qk(0, 1, "q"))
                sched(13, lambda: em_qk(0, 2, "q"))
                sched(21, lambda: em_qk(0, 3, "q"))
                for i in range(4):
                    sched(16 + 2 * i, lambda kc=i: em_qk(1, kc, "k"))
                sched(20, lambda: em_qk(1, 0, "q"))
                for g in range(4):
                    sched(22 + 2 * g, lambda g=g: em_v4(1, g))
                sched(30, lambda: em_qk(1, 1, "q"))
                sched(31, lambda: em_qk(1, 2, "q"))
                sched(32, lambda: em_qk(1, 3, "q"))

                nc.vector.memset(vaug[0][:], 1.0)
                nc.vector.memset(vaug[1][:], 1.0)
                em_qk(0, 0, "k")
                em_qk(0, 0, "q")

                # ---- main software-pipelined loop ---------------------------
                steps = [(b, ic, t, hh) for b in range(B) for ic in range(4)
                         for t in range(4) for hh in range(2)]

                out_acc_of = {}
                pv_counts = {}
                pv_due = {}       # slot -> list of closures
                epi_due = {}      # slot -> list of closures
                carry = None

                def emit_pv(b, ic, t, hh, expT):
                    key = (b, ic)
                    acc = out_acc_of[key]
                    for gg in range(2):
                        jt = 4 * t + 2 * hh + gg
                        for sub in range(4):
                            cnt = pv_counts.get((key, sub), 0) + 1
                            pv_counts[(key, sub)] = cnt
                            nc.tensor.matmul(
                                acc[:, sub * 33:(sub + 1) * 33],
                                lhsT=expT[:, gg * 512 + sub * 128:
                                          gg * 512 + sub * 128 + 128],
                                rhs=vaug[b][:, jt * 33:(jt + 1) * 33],
                                start=False, stop=(cnt == 16),
                                skip_group_check=True)

                def make_epilogue(b, ic):
                    key = (b, ic)
                    acc = out_acc_of[key]
                    st = {}
                    last = (b == B - 1 and ic == 3)

                    def e_evict():
                        st["attn"] = sbepi.tile([128, 132], BF16, tag="attn",
                                                name="attn")
                        nc.vector.tensor_copy(st["attn"][:], acc[:, 0:132])
                        if dbg2_e is not None and (b, ic) == (0, 0):
                            nc.sync.dma_start(dbg2_e[:], st["attn"][:])

                    def e_transpose():
                        st["epi"] = epi_pool.tile([128, 1024], BF16, tag="epi",
                                                  name="epi")
                        outT = st["epi"][0:DH + 1, 0:512]
                        for it in range(4):
                            nc.tensor.transpose(
                                outT[:, it * 128:(it + 1) * 128],
                                st["attn"][:, it * 33:(it + 1) * 33], ident[:])

                    def e_outT():
                        st["outTs"] = sbepi.tile([DH + 1, 512], BF16, tag="outTs",
                                                 name="outTs")
                        nc.vector.tensor_copy(st["outTs"][:],
                                              st["epi"][0:DH + 1, 0:512])
                        st["osb"] = sbepi.tile([128, 4 * C], BF16, tag="osb",
                                               name="osb")

                    def e_half(half):
                        def fn():
                            if half == 0:
                                st["rec"] = sbepi.tile([128, 4], F32, tag="rec",
                                                       name="rec")
                                den = st["attn"][:].rearrange(
                                    "p (s c) -> p s c", s=4)[:, :, 32]
                                nc.vector.reciprocal(st["rec"][:], den)
                                proj = acc[:, 256:512]
                            else:
                                proj = st["epi"][:].bitcast(F32)[:, 256:512]
                            for u in range(2):
                                it = 2 * half + u
                                nc.tensor.matmul(
                                    proj[:, u * C:(u + 1) * C],
                                    lhsT=st["outTs"][:, it * 128:(it + 1) * 128],
                                    rhs=waug[:, 0:C], start=True, stop=True)
                            pv = proj.rearrange("p (f c) -> p f c", f=2)
                            ov = st["osb"][:].rearrange("p (f c) -> p f c", f=4)
                            for u in range(2):
                                it = 2 * half + u
                                nc.vector.tensor_scalar_mul(
                                    ov[:, it, :], pv[:, u, :],
                                    st["rec"][:, it:it + 1])
                            if last:
                                r0 = b * N + ic * 512 + half * 256
                                dst = out_e[r0:r0 + 256, :].rearrange(
                                    "(it p) c -> p it c", it=2)
                                src = st["osb"][:, half * 256:half * 256 + 256]
                                nc.sync.dma_start(dst, src.rearrange(
                                    "p (it c) -> p it c", it=2))
                        return fn

                    def e_store():
                        r0 = b * N + ic * 512
                        dst = out_e[r0:r0 + 512, :].rearrange(
                            "(it p) c -> p it c", it=4)
                        nc.sync.dma_start(dst, st["osb"][:].rearrange(
                            "p (it c) -> p it c", it=4))

                    def e_tp_outT():
                        e_transpose()
                        e_outT()
                    ops = [e_evict, e_tp_outT, e_half(0), e_half(1)]
                    if not last:
                        ops.append(e_store)
                    return ops

                def emit_scores(s2):
                    b2, ic2, t2, hh2 = steps[s2]
                    sc = score_pool.tile([128, 1024], F32, name="score",
                                         tag="score")
                    k7 = ic2 - t2 + 3
                    pt, P = k7 // 2, 64 * (k7 % 2)
                    for gg in range(2):
                        g = 2 * hh2 + gg
                        jt = 4 * t2 + g
                        nc.tensor.matmul(
                            sc[:, gg * 512:(gg + 1) * 512],
                            lhsT=k8[b2][:, jt * 128:(jt + 1) * 128],
                            rhs=q8[b2][:, ic2 * 512:(ic2 + 1) * 512],
                            start=True, stop=False,
                            skip_group_check=True, tile_position=(0, 0))
                        nc.tensor.matmul(
                            sc[:, gg * 512:(gg + 1) * 512],
                            lhsT=lh[P:P + 64, :].rearrange(
                                "k (p m) -> k p m", p=2),
                            rhs=slab[P:P + 64,
                                     pt * 4096 + g * 1024:
                                     pt * 4096 + (g + 1) * 1024].rearrange(
                                "k (p n) -> k p n", p=2),
                            start=False, stop=True, perf_mode=DR,
                            skip_group_check=True, tile_position=(P, 0))
                    return sc

                sc_of = {}
                for s, (b, ic, t, hh) in enumerate(steps):
                    pos = 2 * t + hh
                    chunk = s // 8
                    if pos == 0:
                        acc0 = out_pool.tile(
                            [128, 512], F32, name="out_acc", tag="out_acc")
                        out_acc_of[(b, ic)] = acc0
                        nc.vector.memset(acc0[:, 0:132], 0.0)
                    for fn in ph1_sched.pop(s, ()):
                        fn()

                    # exp of step s-1 must precede scores(s+1): they reuse
                    # the same double-buffered PSUM bank (reader before writer)
                    if carry is not None:
                        carry()
                    if s == 0:
                        sc_of[0] = emit_scores(0)
                    if s + 1 < len(steps):
                        sc_of[s + 1] = emit_scores(s + 1)
                    sc = sc_of.pop(s)

                    # due PV + epilogue drips
                    for fn in pv_due.pop(s, ()):
                        fn()
                    for fn in epi_due.pop(s, ()):
                        fn()

                    def make_exp(b=b, ic=ic, t=t, hh=hh, sc=sc, s=s, pos=pos,
                                 chunk=chunk):
                        def fn():
                            expT = sbexp.tile([128, 1024], BF16, tag="expT",
                                              name="expT")
                            if pos in POOL_POS.get(chunk, ()):
                                scb = sbcp.tile([128, 1024], BF16, tag="scb",
                                                name="scb")
                                nc.vector.tensor_copy(scb[:], sc[:])
                                nc.gpsimd.tensor_tensor(expT[:], base[:], scb[:],
                                                        mybir.AluOpType.pow)
                                due = (chunk + 1) * 8 + 1   # after chunk ends
                            else:
                                nc.scalar.activation(expT[:], sc[:], EXPF)
                                due = s + 2 if s < 55 else s + 1
                            pv_due.setdefault(min(due, len(steps) - 1), []).append(
                                lambda: emit_pv(b, ic, t, hh, expT))
                        return fn

                    carry = make_exp()

                    if pos == 7:
                        ops = make_epilogue(b, ic)
                        if s == len(steps) - 1:
                            epi_due.setdefault(len(steps) - 1, []).extend(ops)
                        else:
                            for i, op in enumerate(ops):
                                epi_due.setdefault(s + 2 + i, []).append(op)

                carry()
                # flush anything scheduled at/after the last slot
                for slot in sorted(pv_due):
                    for fn in pv_due[slot]:
                        fn()
                pv_due.clear()
                for slot in sorted(epi_due):
                    for fn in epi_due[slot]:
                        fn()
                epi_due.clear()
                if dbg_e is not None:
                    nc.sync.dma_start(dbg_e[:, 0:N], q8[0][:])
                    nc.sync.dma_start(dbg_e[:, N:2 * N], k8[0][:])

    nc.compile()
    return nc


_NC = None


def _get_nc():
    global _NC
    if _NC is None:
        _NC = _build()
    return _NC


# ---------------------------------------------------------------------------
# host side
# ---------------------------------------------------------------------------


def _prep_in_maps(x, w_qkv, rel_table, w_out, b_out):
    x = np.asarray(x, np.float32)
    w_qkv = np.asarray(w_qkv, np.float32)
    rel_table = np.asarray(rel_table, np.float32)
    w_out = np.asarray(w_out, np.float32)
    b_out = np.asarray(b_out, np.float32)

    # q/k each carry dh^-0.25 so the product gives the dh^-0.5 score scale
    qk_scale = float(DH ** -0.25)
    xt = np.ascontiguousarray(x.transpose(2, 0, 1).reshape(C, B * N)).astype(NPBF16)

    ident = np.eye(128, dtype=np.float32).astype(NPBF16)
    L = np.zeros((64, 2, 128), np.float32)
    for j in range(128):
        L[j % 64, j // 64, j] = 1.0
    lh = np.vstack([L.reshape(64, 256), L.reshape(64, 256)]).astype(NPFP8)

    in_maps = []
    for hc in range(NCORES):
        w3 = np.concatenate([
            w_qkv[:, hc * DH:(hc + 1) * DH] * qk_scale,
            w_qkv[:, 256 + hc * DH: 256 + (hc + 1) * DH] * qk_scale,
            w_qkv[:, 512 + hc * DH: 512 + (hc + 1) * DH],
        ], axis=1).astype(NPBF16)
        waug = np.zeros((DH + 1, C + 1), np.float32)
        waug[0:DH, 0:C] = w_out[hc * DH:(hc + 1) * DH, :]
        if hc == 0:
            waug[DH, 0:C] = b_out
        waug[DH, C] = 1.0
        b7 = rel_table[:, hc][_IDX7]                     # [7, 4, 128, 512]
        b7r = b7.reshape(7, 4, 2, 64, 512)               # (k7, g, ph, kx, i)
        slab = np.zeros((128, 4, 4, 2, 512), np.float32)
        for k7 in range(7):
            P = 64 * (k7 % 2)
            slab[P:P + 64, k7 // 2] = b7r[k7].transpose(2, 0, 1, 3)
        in_maps.append({
            "xt": xt,
            "w3": np.ascontiguousarray(w3),
            "waug": waug.astype(NPBF16),
            "ident": ident,
            "lh": lh,
            "slab": np.ascontiguousarray(slab.reshape(128, 4 * 4096)).astype(NPFP8),
        })
    return in_maps


def _run(in_maps, **kwargs):
    nc = _get_nc()
    return run_bass_kernel_spmd(nc, in_maps, core_ids=list(range(NCORES)), **kwargs)


def kernel(x, w_qkv, rel_table, w_out, b_out, d=None, h=None, w=None):
    in_maps = _prep_in_maps(x, w_qkv, rel_table, w_out, b_out)
    res = _run(in_maps)
    acc = np.zeros((B * N, C), np.float64)
    for i in range(NCORES):
        acc += res.results[i]["out"].astype(np.float64)
    return acc.reshape(B, N, C).astype(np.float32)
